# revision 34
# baseline (speedup 1.0000x reference)
"""CPAB warp kernel for Trainium2, 8-core data-parallel.

Math: theta = mean_S(input_seq) @ W_loc + b_loc; A = (theta @ basis.T) -> per-cell
affine velocity v(x) = a_c x + b_c (continuous PWL, 64 cells); gamma = 50 Euler
steps of x += v(x)*dt from the uniform grid (S=4096 points in [0,1]).

Facts this kernel exploits (verified against the reference numerics):
 - Cell boundaries fall exactly at s = 64*c: each cell owns 64 consecutive grid
   points; max total drift ~4.8 grid spacings, so only the E=8 outermost points
   per cell side can ever cross a boundary, and never beyond +-1 cell.
 - Within a cell the Euler recurrence is affine: x' = alpha*x + beta
   (alpha = 1+a*dt, beta = b*dt), so the never-crossing trajectory is
   x_t = alpha^t x0 + h_t. A point's crossing indicator is monotone in t
   (1-D autonomous flow), so the crossing step k = #(t: not crossed) and
   afterwards the point follows the DESTINATION cell's affine recurrence:
     x50 = u'*(u*x0 + S*bd) + S'*bd'
   u = alpha^k = 1+em, S = em/(alpha-1), em = expm1(k*log1p(ad)) computed by
   short polynomial series on DVE (|k*ln alpha| <= ~0.1), exact as ad->0
   (no branching needed). Same for u', S' with 50-k, ad'. Bulk points are the
   k=50 case of the same formula (u'=1, S'=0).
 - "Crossed at t" is detected in PASS layout (partition = (row2, cell)):
   right: -x0R >= tr1_t, left: tr2_t >= -x0L, where tr1/tr2 = (h_t - knot)/g_t.
   k and the per-cell constants move to edge layout via exact 0/1 selector
   matmuls (bf16; k <= 50 and selectors are exact).

Layouts: pass layout partition = 64h+c (2 rows x 64 cells); edge layout
partition p = 16r+cq, free = (c4, side, e), c = 4cq+c4; output grow layout
partition p = 16r+cq, free = (c4, j<64) which flattens to gamma rows so the
whole output is ONE contiguous 128KB store (1KB partition lines).

Pipeline per row: one 2MB HWDGE DMA (16KB contiguous partition lines), DVE
fold 32->16 (f32) and 16->8 (straight to bf16; partials tolerate bf16 since
theta errors enter gamma only through the ~1e-3 warp displacement), then 8
bf16 PE matmuls against ones/S accumulating the partition sum in PSUM.
Constants ride the Scalar-engine HWDGE queue so the Sync queue streams
input_seq back-to-back.
"""

import numpy as np

B, S, D = 64, 4096, 128
NCELLS = 64
NSTEPS = 50
DT = 1.0 / NSTEPS
DTH = NCELLS - 1  # 63
NCORES = 8
R = B // NCORES  # 8 rows per core
NPASS = R // 2  # 4 passes of 2 rows
E = 8  # edge points per cell side
NT = S // 128  # 32 s-tiles per row

_CACHE = {}


def _build_program():
    import concourse.bass as bass
    import concourse.bacc as bacc
    import concourse.tile as tile
    from concourse import mybir

    alu = mybir.AluOpType
    f32 = mybir.dt.float32
    bf16 = mybir.dt.bfloat16

    nc = bacc.Bacc("TRN2", target_bir_lowering=False, debug=False, enable_asserts=False)

    seq = nc.dram_tensor("seq", [R, S, D], f32, kind="ExternalInput").ap()
    # all [128, x] constants packed into one DMA; [63, x] into another
    pk = nc.dram_tensor("pk", [128, 969], f32, kind="ExternalInput").ap()
    gamma = nc.dram_tensor("gamma", [R, S], f32, kind="ExternalOutput").ap()

    with tile.TileContext(nc) as tc:
        with (
            tc.tile_pool(name="const", bufs=1) as p_const,
            tc.tile_pool(name="seqp", bufs=5) as p_seq,
            tc.tile_pool(name="red", bufs=2) as p_red,
            tc.tile_pool(name="meanps", bufs=1, space=bass.MemorySpace.PSUM) as p_mps,
            tc.tile_pool(name="passps", bufs=1, space=bass.MemorySpace.PSUM) as p_pps,
            tc.tile_pool(name="kegps", bufs=1, space=bass.MemorySpace.PSUM) as p_kps,
            tc.tile_pool(name="sb", bufs=1) as p_sb,
            tc.tile_pool(name="tbl", bufs=1) as p_tbl,
            tc.tile_pool(name="cmp", bufs=2) as p_cmp,
            tc.tile_pool(name="fin", bufs=1) as p_fin,
        ):
            # ---- row 0 DMA first: own the sync queue from t=0 ----
            seq_tiles = []

            def row_dma(r):
                seq_t = p_seq.tile([128, NT, D], f32, tag="seq", name=f"seq{r}")
                sv = seq[r].rearrange("(p n) d -> p n d", p=128)
                nc.sync.dma_start(seq_t[:, 0 : NT // 2, :], sv[:, 0 : NT // 2, :])
                nc.sync.dma_start(seq_t[:, NT // 2 : NT, :], sv[:, NT // 2 : NT, :])
                seq_tiles.append(seq_t)

            row_dma(0)

            # ---- constants: two packed DMAs on the Scalar HWDGE queue ----
            pk_sb = p_const.tile([128, 969], f32, tag="pk")
            nc.scalar.dma_start(pk_sb[:], pk)
            x0_sb = pk_sb[:, 0:64]
            x0g_sb = pk_sb[:, 64:320].rearrange("p (c f e) -> p c f e", c=4, f=8)
            tk_sb = pk_sb[:, 320:322]
            wbs_sb = pk_sb[:, 322:706]
            ones_sb = pk_sb[:, 706:707]
            eabs_sb = pk_sb[:, 707:963]
            bcons_sb = pk_sb[:, 963:969]

            # bf16 copies of matmul operands on ACT (selectors exact in bf16);
            # neither DVE nor GpSimd is blocked behind const loads this way
            wbs_bf = p_const.tile([128, 6 * 64], bf16, tag="wbsbf")
            eabs_bf = p_const.tile([128, 8 * 32], bf16, tag="eabsbf")
            ones_bf = p_const.tile([128, 1], bf16, tag="onesbf")
            with nc.allow_low_precision("theta pipeline tolerates bf16"):
                nc.scalar.copy(wbs_bf[:], wbs_sb)
                nc.scalar.copy(eabs_bf[:], eabs_sb)
                nc.scalar.copy(ones_bf[:], ones_sb)
            negx0 = p_const.tile([128, 2, E], f32, tag="negx0")
            nc.scalar.mul(negx0[:, 0, :], x0_sb[:, 64 - E : 64], -1.0)
            nc.scalar.mul(negx0[:, 1, :], x0_sb[:, 0:E], -1.0)
            zrep = p_const.tile([128, NSTEPS + 1], f32, tag="zrep")
            nc.gpsimd.memset(zrep[:], 0.0)

            # gh tiles preset up front (removes 2 memsets from each pass chain)
            gh_tiles = []
            for g in range(NPASS):
                gh = p_tbl.tile(
                    [128, 2, NSTEPS + 1], f32, tag=f"gh{g}", name=f"gh{g}"
                )
                nc.gpsimd.memset(gh[:, 0, 0:1], 1.0)
                nc.gpsimd.memset(gh[:, 1, 0:1], 0.0)
                gh_tiles.append(gh)

            mean_ps = p_mps.tile([128, R], f32, tag="meanps")
            mean_bf = p_sb.tile([128, R], bf16, tag="meanbf")
            # expanded edge tables: cols 0:8 k(e), 8 ad, 9 bd, 10 ad', 11 bd'
            keg = p_sb.tile([128, 8, 12], f32, tag="keg")
            keg_ps = p_kps.tile([128, 8, 12], f32, tag="kegps")

            # ---- phase 1: DVE folds + bf16 partition-sum matmuls ----
            def do_row(r):
                if r > 0:
                    row_dma(r)
                seq_t = seq_tiles[r]
                # fold 32 -> 8 s-tiles: quarter-sums split across GpSimd + DVE
                s1 = p_red.tile([128, 2, NT // 4, D], f32, tag="s1", name=f"s1_{r}")
                nc.gpsimd.tensor_tensor(
                    out=s1[:, 0, :, :], in0=seq_t[:, 0 : NT // 4, :],
                    in1=seq_t[:, NT // 4 : NT // 2, :], op=alu.add,
                )
                nc.vector.tensor_tensor(
                    out=s1[:, 1, :, :], in0=seq_t[:, NT // 2 : 3 * NT // 4, :],
                    in1=seq_t[:, 3 * NT // 4 : NT, :], op=alu.add,
                )
                s2f = p_red.tile([128, NT // 4, D], f32, tag="s2f", name=f"s2f{r}")
                nc.vector.tensor_tensor(
                    out=s2f[:], in0=s1[:, 0, :, :], in1=s1[:, 1, :, :], op=alu.add
                )
                s2b = p_red.tile([128, NT // 4, D], bf16, tag="s2b", name=f"s2b{r}")
                with nc.allow_low_precision("mean partials tolerate bf16"):
                    nc.scalar.copy(s2b[:], s2f[:])
                for i in range(NT // 4):
                    nc.tensor.matmul(
                        mean_ps[:, r : r + 1], s2b[:, i, :], ones_bf[:],
                        start=(i == 0), stop=(i == NT // 4 - 1),
                    )

            def do_pass(g):
                with nc.allow_low_precision("theta pipeline tolerates bf16"):
                    nc.vector.tensor_copy(
                        mean_bf[:, 2 * g : 2 * g + 2], mean_ps[:, 2 * g : 2 * g + 2]
                    )
                # per-(h,c) consts a_cur..b_prv straight from the mean:
                # cons_q = WBS_q^T mean + bcons_q, WBS_q = (W@basis.T)@sel_q (host)
                cps = p_pps.tile([128, 6], f32, tag="cps", name=f"cps{g}")
                for h in range(2):
                    for q in range(6):
                        nc.tensor.matmul(
                            cps[64 * h : 64 * h + 64, q : q + 1],
                            wbs_bf[:, 64 * q : 64 * q + 64],
                            mean_bf[:, 2 * g + h : 2 * g + h + 1],
                            start=True, stop=True,
                        )
                cons = p_tbl.tile([128, 6], f32, tag=f"cons{g}", name=f"cons{g}")
                nc.vector.tensor_tensor(
                    out=cons[:], in0=cps[:], in1=bcons_sb, op=alu.add
                )
                a_cur, b_cur = cons[:, 0:1], cons[:, 1:2]

                sc = p_tbl.tile([128, 2], f32, tag=f"sc{g}", name=f"sc{g}")
                alpha = sc[:, 0:1]
                nc.vector.tensor_scalar(
                    out=alpha, in0=a_cur, scalar1=float(DT), scalar2=1.0,
                    op0=alu.mult, op1=alu.add,
                )
                arep = p_tbl.tile([128, NSTEPS + 1], f32, tag=f"arep{g}", name=f"arep{g}")
                nc.vector.tensor_scalar(
                    out=arep[:], in0=zrep[:], scalar1=alpha, scalar2=None, op0=alu.add
                )
                brep = p_tbl.tile([128, NSTEPS + 1], f32, tag=f"brep{g}", name=f"brep{g}")
                nc.vector.tensor_scalar(
                    out=brep[:], in0=zrep[:], scalar1=b_cur, scalar2=float(DT),
                    op0=alu.add, op1=alu.mult,
                )
                gh = gh_tiles[g]
                gt, ht = gh[:, 0, :], gh[:, 1, :]
                nc.vector.tensor_tensor_scan(
                    out=gt[:, 1 : NSTEPS + 1], data0=arep[:, 0:NSTEPS],
                    data1=zrep[:, 0:NSTEPS], initial=1.0, op0=alu.mult, op1=alu.add,
                )
                nc.vector.tensor_tensor_scan(
                    out=ht[:, 1 : NSTEPS + 1], data0=arep[:, 0:NSTEPS],
                    data1=brep[:, 0:NSTEPS], initial=0.0, op0=alu.mult, op1=alu.add,
                )
                rg = p_tbl.tile([128, NSTEPS], f32, tag=f"rg{g}", name=f"rg{g}")
                nc.vector.reciprocal(rg[:], gt[:, 0:NSTEPS])

                # tr1_t = (h_t - t+)/g_t ; tr2_t = (h_t - t-)/g_t
                tr = p_tbl.tile([128, 2, NSTEPS], f32, tag=f"tr{g}", name=f"tr{g}")
                nc.vector.scalar_tensor_tensor(
                    out=tr[:, 0, :], in0=ht[:, 0:NSTEPS], scalar=tk_sb[:, 1:2],
                    in1=rg[:], op0=alu.subtract, op1=alu.mult,
                )
                nc.vector.scalar_tensor_tensor(
                    out=tr[:, 1, :], in0=ht[:, 0:NSTEPS], scalar=tk_sb[:, 0:1],
                    in1=rg[:], op0=alu.subtract, op1=alu.mult,
                )

                # crossing counts in pass layout; not-crossed_R: -x0R >= tr1
                kprep = p_tbl.tile([128, 2, 12], bf16, tag=f"kp{g}", name=f"kp{g}")
                with nc.allow_low_precision("cell consts tolerate bf16"):
                    # consts first: their expansion (and la/rad prep) can run
                    # while the crossing counts are still being reduced
                    nc.vector.tensor_scalar(
                        out=kprep[:, :, 8:10],
                        in0=cons[:, 0:2].rearrange("p (o c) -> p o c", o=1).broadcast_to(
                            [128, 2, 2]
                        ),
                        scalar1=float(DT), scalar2=None, op0=alu.mult,
                    )
                    nc.vector.tensor_scalar(
                        out=kprep[:, :, 10:12],
                        in0=cons[:, 2:6].rearrange("p (c s) -> p s c", c=2),
                        scalar1=float(DT), scalar2=None, op0=alu.mult,
                    )
                # one fused compare: plane R counts crossed (-x0R <= tr1),
                # plane L counts not-crossed (-x0L <= tr2); R fixed to 50-k
                # after expansion
                cmpf = p_cmp.tile([128, 2, E, NSTEPS], f32, tag="cmp", name=f"cmp{g}")
                nc.vector.tensor_tensor(
                    out=cmpf[:],
                    in0=negx0[:].rearrange("p s (e o) -> p s e o", o=1).broadcast_to(
                        [128, 2, E, NSTEPS]
                    ),
                    in1=tr[:].rearrange("p s (o t) -> p s o t", o=1).broadcast_to(
                        [128, 2, E, NSTEPS]
                    ),
                    op=alu.is_le,
                )
                with nc.allow_low_precision("k <= 50 exact in bf16"):
                    # one reduce for both sides: R counts land in kprep[:,0,0:8],
                    # L in kprep[:,1,0:8] (input tile holds R then L planes)
                    nc.vector.tensor_reduce(
                        out=kprep[:, :, 0:E],
                        in_=cmpf[:],
                        axis=mybir.AxisListType.X, op=alu.add,
                    )

                # expand (k, consts) into edge layout via exact 0/1 matmuls
                for ch in range(8):
                    side = ch % 2  # 0=L, 1=R
                    nc.tensor.matmul(
                        keg_ps[32 * g : 32 * g + 32, ch, :],
                        eabs_bf[:, 32 * ch : 32 * ch + 32],
                        kprep[:, 1 - side, :],
                        start=True, stop=True, tile_position=(0, 32 * g),
                    )
                nc.scalar.copy(
                    keg[32 * g : 32 * g + 32, :, :], keg_ps[32 * g : 32 * g + 32, :, :]
                )

            # passes delayed one row-group: every pass consumes ~10us-old
            # data, so its cross-engine chain never stalls the fold stream
            for r in range(R):
                do_row(r)
                if r == 3:
                    do_pass(0)
                elif r == 5:
                    do_pass(1)
                elif r == 7:
                    do_pass(2)
                    do_pass(3)

            # ---- closed-form finals on the edge tile ----
            # R channels (odd) counted crossed steps: k = 50 - count
            nc.vector.tensor_scalar(
                out=keg[:, 1:8:2, 0:E], in0=keg[:, 1:8:2, 0:E], scalar1=-1.0,
                scalar2=float(NSTEPS), op0=alu.mult, op1=alu.add,
            )
            kf = keg[:, :, 0:E]
            adv = keg[:, :, 8]
            bd_b = keg[:, :, 9:10].broadcast_to([128, 8, E])
            adpv = keg[:, :, 10]
            bdp_b = keg[:, :, 11:12].broadcast_to([128, 8, E])

            prep = p_fin.tile([128, 4, 8], f32, tag="prep")
            la, lap, rad, radp = (
                prep[:, 0, :], prep[:, 1, :], prep[:, 2, :], prep[:, 3, :],
            )
            t8 = p_fin.tile([128, 8], f32, tag="t8")

            def ln1p(out, x):  # ln(1+x) ~ x*(1 - x/2), |x| <= ~2e-3
                nc.vector.tensor_scalar(
                    out=t8[:], in0=x, scalar1=-0.5, scalar2=1.0,
                    op0=alu.mult, op1=alu.add,
                )
                nc.vector.tensor_tensor(out=out, in0=t8[:], in1=x, op=alu.mult)

            ln1p(la, adv)
            ln1p(lap, adpv)
            nc.vector.reciprocal(rad, adv)
            nc.vector.reciprocal(radp, adpv)

            def bview(x, n=E):  # [128, m] -> [128, m, n] broadcast
                return x.rearrange("p (c o) -> p c o", o=1).broadcast_to(
                    [128, x.shape[1], n]
                )

            tt = nc.vector.tensor_tensor
            ts = nc.vector.tensor_scalar

            def expm1s(out, y, tmp):  # y*(1+y/2*(1+y/3*(1+y/4))), |y| <= ~0.1
                ts(out=tmp[:], in0=y[:], scalar1=0.25, scalar2=1.0,
                   op0=alu.mult, op1=alu.add)
                tt(out=tmp[:], in0=tmp[:], in1=y[:], op=alu.mult)
                ts(out=tmp[:], in0=tmp[:], scalar1=1.0 / 3.0, scalar2=1.0,
                   op0=alu.mult, op1=alu.add)
                tt(out=tmp[:], in0=tmp[:], in1=y[:], op=alu.mult)
                ts(out=tmp[:], in0=tmp[:], scalar1=0.5, scalar2=1.0,
                   op0=alu.mult, op1=alu.add)
                tt(out=out[:], in0=tmp[:], in1=y[:], op=alu.mult)

            # x0 views from the grow-layout grid constant (4D; strided views
            # cannot be flattened, so edge/bulk ops run on 4D access patterns)
            x0e = x0g_sb[:, :, 0:8:7, :]
            x0bulk = x0g_sb[:, :, 1:7, :]

            def v4(a):  # [128, 8, E] contiguous tile -> [128, 4, 2, E] view
                return a.rearrange("p (c f) e -> p c f e", f=2)

            y = p_fin.tile([128, 8, E], f32, tag="y")
            tmp = p_fin.tile([128, 8, E], f32, tag="tmp")
            em = p_fin.tile([128, 8, E], f32, tag="em")
            emp = p_fin.tile([128, 8, E], f32, tag="emp")
            # em chain on DVE; em' chain on GpSimd in parallel
            tt(out=y[:], in0=kf, in1=bview(la), op=alu.mult)
            expm1s(em, y, tmp)
            kc = p_fin.tile([128, 8, E], f32, tag="kc")
            y2 = p_fin.tile([128, 8, E], f32, tag="y2")
            tm2 = p_fin.tile([128, 8, E], f32, tag="tm2")
            spb = p_fin.tile([128, 8, E], f32, tag="spb")
            gt_ = nc.gpsimd.tensor_tensor
            gs_ = nc.gpsimd.tensor_scalar
            gs_(out=kc[:], in0=kf, scalar1=-1.0, scalar2=float(NSTEPS),
                op0=alu.mult, op1=alu.add)
            gt_(out=y2[:], in0=kc[:], in1=bview(lap), op=alu.mult)
            gs_(out=tm2[:], in0=y2[:], scalar1=0.25, scalar2=1.0,
                op0=alu.mult, op1=alu.add)
            gt_(out=tm2[:], in0=tm2[:], in1=y2[:], op=alu.mult)
            gs_(out=tm2[:], in0=tm2[:], scalar1=1.0 / 3.0, scalar2=1.0,
                op0=alu.mult, op1=alu.add)
            gt_(out=tm2[:], in0=tm2[:], in1=y2[:], op=alu.mult)
            gs_(out=tm2[:], in0=tm2[:], scalar1=0.5, scalar2=1.0,
                op0=alu.mult, op1=alu.add)
            gt_(out=emp[:], in0=tm2[:], in1=y2[:], op=alu.mult)
            gt_(out=spb[:], in0=emp[:], in1=bview(radp), op=alu.mult)
            gt_(out=spb[:], in0=spb[:], in1=bdp_b, op=alu.mult)  # S'*bd'

            grow = p_fin.tile([128, 4, 8, E], f32, tag="grow")
            growe = grow[:, :, 0:8:7, :]

            # x50 = (1+em')*((1+em)*x0 + em*rad*bd) + em'*radp*bd'
            P = p_fin.tile([128, 8, E], f32, tag="P")
            tt(out=tmp[:], in0=em[:], in1=bview(rad), op=alu.mult)
            tt(out=tmp[:], in0=tmp[:], in1=bd_b, op=alu.mult)  # S*bd
            tt(out=v4(y[:]), in0=v4(em[:]), in1=x0e, op=alu.mult)
            tt(out=tmp[:], in0=tmp[:], in1=y[:], op=alu.add)
            tt(out=v4(P[:]), in0=v4(tmp[:]), in1=x0e, op=alu.add)  # u*x0 + S*bd
            tt(out=tmp[:], in0=emp[:], in1=P[:], op=alu.mult)
            tt(out=tmp[:], in0=tmp[:], in1=P[:], op=alu.add)  # u'*P
            tt(out=growe, in0=v4(tmp[:]), in1=v4(spb[:]), op=alu.add)

            # bulk = k=50 case per cell: x = (1+em50)*x0 + em50*rad*bd
            la4 = la[:, 0:8:2]
            rad4 = rad[:, 0:8:2]
            bd4 = keg[:, 0:8:2, 9]
            t4 = p_fin.tile([128, 4], f32, tag="t4")
            y4 = p_fin.tile([128, 4], f32, tag="y4")
            em50 = p_fin.tile([128, 4], f32, tag="em50")
            gs_(out=y4[:], in0=la4, scalar1=float(NSTEPS), scalar2=None, op0=alu.mult)
            gs_(out=t4[:], in0=y4[:], scalar1=0.25, scalar2=1.0,
                op0=alu.mult, op1=alu.add)
            gt_(out=t4[:], in0=t4[:], in1=y4[:], op=alu.mult)
            gs_(out=t4[:], in0=t4[:], scalar1=1.0 / 3.0, scalar2=1.0,
                op0=alu.mult, op1=alu.add)
            gt_(out=t4[:], in0=t4[:], in1=y4[:], op=alu.mult)
            gs_(out=t4[:], in0=t4[:], scalar1=0.5, scalar2=1.0,
                op0=alu.mult, op1=alu.add)
            gt_(out=em50[:], in0=t4[:], in1=y4[:], op=alu.mult)
            sbd4 = p_fin.tile([128, 4], f32, tag="sbd4")
            gt_(out=sbd4[:], in0=em50[:], in1=rad4, op=alu.mult)
            gt_(out=sbd4[:], in0=sbd4[:], in1=bd4, op=alu.mult)
            growb = grow[:, :, 1:7, :]

            def b4(x):  # [128, 4] -> [128, 4, 6, E] broadcast
                return x.rearrange("p (c o u) -> p c o u", o=1, u=1).broadcast_to(
                    [128, 4, 6, E]
                )

            tb = p_fin.tile([128, 4, 6, E], f32, tag="tb")
            gt_(out=tb[:], in0=x0bulk, in1=b4(em50[:]), op=alu.mult)
            gt_(out=tb[:], in0=tb[:], in1=x0bulk, op=alu.add)
            gt_(out=growb, in0=tb[:], in1=b4(sbd4[:]), op=alu.add)

            # ---- one contiguous store: grow == gamma rows (sync queue is
            # empty by now; scalar queue still drains const traffic) ----
            nc.sync.dma_start(
                gamma.rearrange("r (q m) -> (r q) m", m=4 * 64),
                grow[:].rearrange("p c f e -> p (c f e)"),
            )

    nc.compile()
    return nc


def _host_constants():
    f32 = np.float32
    grid = np.linspace(0.0, 1.0, S).astype(f32)
    c = np.arange(128, dtype=np.int64) % 64
    x0map = grid[(64 * c)[:, None] + np.arange(64)[None, :]]
    # grow layout: x0g[p, 64*c4 + j] = grid[64*(4*(p%16)+c4) + j]
    cq = np.arange(128, dtype=np.int64) % 16
    cell = 4 * cq[:, None] + np.arange(256)[None, :] // 64
    x0g = grid[64 * cell + np.arange(256)[None, :] % 64]
    tknots = np.stack([c / 64.0, (c + 1) / 64.0], axis=1).astype(f32)
    sel = np.zeros((128, 6 * 64), dtype=f32)
    cc = np.arange(64)
    sel[2 * cc, 0 * 64 + cc] = 1.0  # a_cur
    sel[2 * cc + 1, 1 * 64 + cc] = 1.0  # b_cur
    sel[np.minimum(2 * cc + 2, 126), 2 * 64 + cc] = 1.0  # a_nxt (c=63 -> self)
    sel[np.maximum(2 * cc - 2, 0), 3 * 64 + cc] = 1.0  # a_prv (c=0 -> self)
    sel[np.minimum(2 * cc + 3, 127), 4 * 64 + cc] = 1.0  # b_nxt (c=63 -> self)
    sel[np.maximum(2 * cc - 1, 1), 5 * 64 + cc] = 1.0  # b_prv (c=0 -> self)
    onesS = np.full((128, 1), 1.0 / S, dtype=f32)  # 2^-12, exact

    # expansion selectors: k = h*64 + c (pass layout), m = 16*h + cq (local)
    eabs = np.zeros((128, 8 * 32), dtype=f32)
    for ch in range(8):
        c4 = ch // 2
        for m in range(32):
            h, cq_ = m // 16, m % 16
            k = h * 64 + 4 * cq_ + c4
            eabs[k, 32 * ch + m] = 1.0
    return x0map, x0g, tknots, sel, onesS, eabs


def _in_map(input_seq_slice, W_loc, b_loc, basis, consts):
    f32 = np.float32
    x0map, x0g, tknots, sel, onesS, eabs = consts
    # fold theta->A->selector gathers into one matrix: cons_q = WBS_q^T mean + bc_q
    WB = np.asarray(W_loc, dtype=np.float64) @ np.asarray(basis, dtype=np.float64).T
    bb = np.asarray(b_loc, dtype=np.float64) @ np.asarray(basis, dtype=np.float64).T
    wbs = (WB @ sel.astype(np.float64).reshape(128, 6, 64).transpose(1, 0, 2)).transpose(
        1, 0, 2
    )  # [128, 6, 64]
    bcons_c = bb @ sel.astype(np.float64).reshape(128, 6, 64).transpose(1, 0, 2)  # [6, 64]
    bcons = np.tile(bcons_c.T, (2, 1)).astype(f32)  # [128=(h c), 6]
    return {
        "seq": np.ascontiguousarray(input_seq_slice, dtype=f32),
        "pk": np.ascontiguousarray(
            np.concatenate(
                [
                    x0map, x0g, tknots,
                    wbs.reshape(128, 384).astype(f32), onesS, eabs, bcons,
                ],
                axis=1,
            ).astype(f32)
        ),
    }


def kernel(input_seq, W_loc, b_loc, basis):
    from concourse.bass_utils import run_bass_kernel_spmd

    if "nc" not in _CACHE:
        _CACHE["nc"] = _build_program()
    nc = _CACHE["nc"]
    consts = _host_constants()
    in_maps = [
        _in_map(input_seq[k * R : (k + 1) * R], W_loc, b_loc, basis, consts)
        for k in range(NCORES)
    ]
    res = run_bass_kernel_spmd(nc, in_maps, core_ids=list(range(NCORES)))
    return np.concatenate([r["gamma"] for r in res.results], axis=0)


# revision 35
# speedup vs baseline: 1.0219x; 1.0219x over previous
"""CPAB warp kernel for Trainium2, 8-core data-parallel.

Math: theta = mean_S(input_seq) @ W_loc + b_loc; A = (theta @ basis.T) -> per-cell
affine velocity v(x) = a_c x + b_c (continuous PWL, 64 cells); gamma = 50 Euler
steps of x += v(x)*dt from the uniform grid (S=4096 points in [0,1]).

Facts this kernel exploits (verified against the reference numerics):
 - Cell boundaries fall exactly at s = 64*c: each cell owns 64 consecutive grid
   points; max total drift ~4.8 grid spacings, so only the E=8 outermost points
   per cell side can ever cross a boundary, and never beyond +-1 cell.
 - Within a cell the Euler recurrence is affine: x' = alpha*x + beta
   (alpha = 1+a*dt, beta = b*dt), so the never-crossing trajectory is
   x_t = alpha^t x0 + h_t. A point's crossing indicator is monotone in t
   (1-D autonomous flow), so the crossing step k = #(t: not crossed) and
   afterwards the point follows the DESTINATION cell's affine recurrence:
     x50 = u'*(u*x0 + S*bd) + S'*bd'
   u = alpha^k = 1+em, S = em/(alpha-1), em = expm1(k*log1p(ad)) computed by
   short polynomial series on DVE (|k*ln alpha| <= ~0.1), exact as ad->0
   (no branching needed). Same for u', S' with 50-k, ad'. Bulk points are the
   k=50 case of the same formula (u'=1, S'=0).
 - "Crossed at t" is detected in PASS layout (partition = (row2, cell)):
   right: -x0R >= tr1_t, left: tr2_t >= -x0L, where tr1/tr2 = (h_t - knot)/g_t.
   k and the per-cell constants move to edge layout via exact 0/1 selector
   matmuls (bf16; k <= 50 and selectors are exact).

Layouts: pass layout partition = 64h+c (2 rows x 64 cells); edge layout
partition p = 16r+cq, free = (c4, side, e), c = 4cq+c4; output grow layout
partition p = 16r+cq, free = (c4, j<64) which flattens to gamma rows so the
whole output is ONE contiguous 128KB store (1KB partition lines).

Pipeline per row: one 2MB HWDGE DMA (16KB contiguous partition lines), DVE
fold 32->16 (f32) and 16->8 (straight to bf16; partials tolerate bf16 since
theta errors enter gamma only through the ~1e-3 warp displacement), then 8
bf16 PE matmuls against ones/S accumulating the partition sum in PSUM.
Constants ride the Scalar-engine HWDGE queue so the Sync queue streams
input_seq back-to-back.
"""

import numpy as np

B, S, D = 64, 4096, 128
NCELLS = 64
NSTEPS = 50
DT = 1.0 / NSTEPS
DTH = NCELLS - 1  # 63
NCORES = 8
R = B // NCORES  # 8 rows per core
NPASS = R // 2  # 4 passes of 2 rows
E = 8  # edge points per cell side
NT = S // 128  # 32 s-tiles per row

_CACHE = {}


def _build_program():
    import concourse.bass as bass
    import concourse.bacc as bacc
    import concourse.tile as tile
    from concourse import mybir

    alu = mybir.AluOpType
    f32 = mybir.dt.float32
    bf16 = mybir.dt.bfloat16

    nc = bacc.Bacc("TRN2", target_bir_lowering=False, debug=False, enable_asserts=False)

    seq = nc.dram_tensor("seq", [R, S, D], f32, kind="ExternalInput").ap()
    # all [128, x] constants packed into one DMA; [63, x] into another
    pk = nc.dram_tensor("pk", [128, 969], f32, kind="ExternalInput").ap()
    gamma = nc.dram_tensor("gamma", [R, S], f32, kind="ExternalOutput").ap()

    with tile.TileContext(nc) as tc:
        with (
            tc.tile_pool(name="const", bufs=1) as p_const,
            tc.tile_pool(name="seqp", bufs=5) as p_seq,
            tc.tile_pool(name="red", bufs=2) as p_red,
            tc.tile_pool(name="meanps", bufs=1, space=bass.MemorySpace.PSUM) as p_mps,
            tc.tile_pool(name="passps", bufs=1, space=bass.MemorySpace.PSUM) as p_pps,
            tc.tile_pool(name="kegps", bufs=1, space=bass.MemorySpace.PSUM) as p_kps,
            tc.tile_pool(name="sb", bufs=1) as p_sb,
            tc.tile_pool(name="tbl", bufs=1) as p_tbl,
            tc.tile_pool(name="cmp", bufs=2) as p_cmp,
            tc.tile_pool(name="fin", bufs=1) as p_fin,
        ):
            # ---- row 0 DMA first: own the sync queue from t=0 ----
            seq_tiles = []

            def row_dma(r):
                seq_t = p_seq.tile([128, NT, D], f32, tag="seq", name=f"seq{r}")
                sv = seq[r].rearrange("(p n) d -> p n d", p=128)
                nc.sync.dma_start(seq_t[:, 0 : NT // 2, :], sv[:, 0 : NT // 2, :])
                nc.sync.dma_start(seq_t[:, NT // 2 : NT, :], sv[:, NT // 2 : NT, :])
                seq_tiles.append(seq_t)

            row_dma(0)

            # ---- constants: two packed DMAs on the Scalar HWDGE queue ----
            pk_sb = p_const.tile([128, 969], f32, tag="pk")
            nc.scalar.dma_start(pk_sb[:], pk)
            x0_sb = pk_sb[:, 0:64]
            x0g_sb = pk_sb[:, 64:320].rearrange("p (c f e) -> p c f e", c=4, f=8)
            tk_sb = pk_sb[:, 320:322]
            wbs_sb = pk_sb[:, 322:706]
            ones_sb = pk_sb[:, 706:707]
            eabs_sb = pk_sb[:, 707:963]
            bcons_sb = pk_sb[:, 963:969]

            # bf16 copies of matmul operands on ACT (selectors exact in bf16);
            # neither DVE nor GpSimd is blocked behind const loads this way
            wbs_bf = p_const.tile([128, 6 * 64], bf16, tag="wbsbf")
            eabs_bf = p_const.tile([128, 8 * 32], bf16, tag="eabsbf")
            ones_bf = p_const.tile([128, 1], bf16, tag="onesbf")
            with nc.allow_low_precision("theta pipeline tolerates bf16"):
                nc.scalar.copy(wbs_bf[:], wbs_sb)
                nc.scalar.copy(eabs_bf[:], eabs_sb)
                nc.scalar.copy(ones_bf[:], ones_sb)
            negx0 = p_const.tile([128, 2, E], f32, tag="negx0")
            nc.scalar.mul(negx0[:, 0, :], x0_sb[:, 64 - E : 64], -1.0)
            nc.scalar.mul(negx0[:, 1, :], x0_sb[:, 0:E], -1.0)
            zrep = p_const.tile([128, NSTEPS + 1], f32, tag="zrep")
            nc.gpsimd.memset(zrep[:], 0.0)

            # gh tiles preset up front (removes 2 memsets from each pass chain)
            gh_tiles = []
            for g in range(NPASS):
                gh = p_tbl.tile(
                    [128, 2, NSTEPS + 1], f32, tag=f"gh{g}", name=f"gh{g}"
                )
                nc.gpsimd.memset(gh[:, 0, 0:1], 1.0)
                nc.gpsimd.memset(gh[:, 1, 0:1], 0.0)
                gh_tiles.append(gh)

            mean_ps = p_mps.tile([128, R], f32, tag="meanps")
            mean_bf = p_sb.tile([128, R], bf16, tag="meanbf")
            # expanded edge tables: cols 0:8 k(e), 8 ad, 9 bd, 10 ad', 11 bd'
            keg = p_sb.tile([128, 8, 12], f32, tag="keg")
            keg_ps = p_kps.tile([128, 8, 12], f32, tag="kegps")

            # ---- phase 1: DVE folds + bf16 partition-sum matmuls ----
            def do_row(r):
                if r > 0:
                    row_dma(r)
                seq_t = seq_tiles[r]
                # fold 32 -> 8 s-tiles: quarter-sums split across GpSimd + DVE
                s1 = p_red.tile([128, 2, NT // 4, D], f32, tag="s1", name=f"s1_{r}")
                nc.gpsimd.tensor_tensor(
                    out=s1[:, 0, :, :], in0=seq_t[:, 0 : NT // 4, :],
                    in1=seq_t[:, NT // 4 : NT // 2, :], op=alu.add,
                )
                nc.vector.tensor_tensor(
                    out=s1[:, 1, :, :], in0=seq_t[:, NT // 2 : 3 * NT // 4, :],
                    in1=seq_t[:, 3 * NT // 4 : NT, :], op=alu.add,
                )
                s2f = p_red.tile([128, NT // 4, D], f32, tag="s2f", name=f"s2f{r}")
                nc.vector.tensor_tensor(
                    out=s2f[:], in0=s1[:, 0, :, :], in1=s1[:, 1, :, :], op=alu.add
                )
                s2b = p_red.tile([128, NT // 4, D], bf16, tag="s2b", name=f"s2b{r}")
                with nc.allow_low_precision("mean partials tolerate bf16"):
                    nc.scalar.copy(s2b[:], s2f[:])
                for i in range(NT // 4):
                    nc.tensor.matmul(
                        mean_ps[:, r : r + 1], s2b[:, i, :], ones_bf[:],
                        start=(i == 0), stop=(i == NT // 4 - 1),
                    )

            def do_pass(g):
                with nc.allow_low_precision("theta pipeline tolerates bf16"):
                    nc.vector.tensor_copy(
                        mean_bf[:, 2 * g : 2 * g + 2], mean_ps[:, 2 * g : 2 * g + 2]
                    )
                # per-(h,c) consts a_cur..b_prv straight from the mean:
                # cons_q = WBS_q^T mean + bcons_q, WBS_q = (W@basis.T)@sel_q (host)
                cps = p_pps.tile([128, 6], f32, tag="cps", name=f"cps{g}")
                for h in range(2):
                    for q in range(6):
                        nc.tensor.matmul(
                            cps[64 * h : 64 * h + 64, q : q + 1],
                            wbs_bf[:, 64 * q : 64 * q + 64],
                            mean_bf[:, 2 * g + h : 2 * g + h + 1],
                            start=True, stop=True,
                        )
                cons = p_tbl.tile([128, 6], f32, tag=f"cons{g}", name=f"cons{g}")
                nc.vector.tensor_tensor(
                    out=cons[:], in0=cps[:], in1=bcons_sb, op=alu.add
                )
                a_cur, b_cur = cons[:, 0:1], cons[:, 1:2]

                sc = p_tbl.tile([128, 2], f32, tag=f"sc{g}", name=f"sc{g}")
                alpha = sc[:, 0:1]
                nc.vector.tensor_scalar(
                    out=alpha, in0=a_cur, scalar1=float(DT), scalar2=1.0,
                    op0=alu.mult, op1=alu.add,
                )
                arep = p_tbl.tile([128, NSTEPS + 1], f32, tag=f"arep{g}", name=f"arep{g}")
                nc.vector.tensor_scalar(
                    out=arep[:], in0=zrep[:], scalar1=alpha, scalar2=None, op0=alu.add
                )
                brep = p_tbl.tile([128, NSTEPS + 1], f32, tag=f"brep{g}", name=f"brep{g}")
                nc.vector.tensor_scalar(
                    out=brep[:], in0=zrep[:], scalar1=b_cur, scalar2=float(DT),
                    op0=alu.add, op1=alu.mult,
                )
                gh = gh_tiles[g]
                gt, ht = gh[:, 0, :], gh[:, 1, :]
                nc.vector.tensor_tensor_scan(
                    out=gt[:, 1 : NSTEPS + 1], data0=arep[:, 0:NSTEPS],
                    data1=zrep[:, 0:NSTEPS], initial=1.0, op0=alu.mult, op1=alu.add,
                )
                nc.vector.tensor_tensor_scan(
                    out=ht[:, 1 : NSTEPS + 1], data0=arep[:, 0:NSTEPS],
                    data1=brep[:, 0:NSTEPS], initial=0.0, op0=alu.mult, op1=alu.add,
                )
                rg = p_tbl.tile([128, NSTEPS], f32, tag=f"rg{g}", name=f"rg{g}")
                nc.vector.reciprocal(rg[:], gt[:, 0:NSTEPS])

                # tr1_t = (h_t - t+)/g_t ; tr2_t = (h_t - t-)/g_t
                tr = p_tbl.tile([128, 2, NSTEPS], f32, tag=f"tr{g}", name=f"tr{g}")
                nc.vector.scalar_tensor_tensor(
                    out=tr[:, 0, :], in0=ht[:, 0:NSTEPS], scalar=tk_sb[:, 1:2],
                    in1=rg[:], op0=alu.subtract, op1=alu.mult,
                )
                nc.vector.scalar_tensor_tensor(
                    out=tr[:, 1, :], in0=ht[:, 0:NSTEPS], scalar=tk_sb[:, 0:1],
                    in1=rg[:], op0=alu.subtract, op1=alu.mult,
                )

                # crossing counts in pass layout; not-crossed_R: -x0R >= tr1
                kprep = p_tbl.tile([128, 2, 12], bf16, tag=f"kp{g}", name=f"kp{g}")
                with nc.allow_low_precision("cell consts tolerate bf16"):
                    # consts first: their expansion (and la/rad prep) can run
                    # while the crossing counts are still being reduced
                    nc.vector.tensor_scalar(
                        out=kprep[:, :, 8:10],
                        in0=cons[:, 0:2].rearrange("p (o c) -> p o c", o=1).broadcast_to(
                            [128, 2, 2]
                        ),
                        scalar1=float(DT), scalar2=None, op0=alu.mult,
                    )
                    nc.vector.tensor_scalar(
                        out=kprep[:, :, 10:12],
                        in0=cons[:, 2:6].rearrange("p (c s) -> p s c", c=2),
                        scalar1=float(DT), scalar2=None, op0=alu.mult,
                    )
                # one fused compare: plane R counts crossed (-x0R <= tr1),
                # plane L counts not-crossed (-x0L <= tr2); R fixed to 50-k
                # after expansion
                cmpf = p_cmp.tile([128, 2, E, NSTEPS], f32, tag="cmp", name=f"cmp{g}")
                nc.vector.tensor_tensor(
                    out=cmpf[:],
                    in0=negx0[:].rearrange("p s (e o) -> p s e o", o=1).broadcast_to(
                        [128, 2, E, NSTEPS]
                    ),
                    in1=tr[:].rearrange("p s (o t) -> p s o t", o=1).broadcast_to(
                        [128, 2, E, NSTEPS]
                    ),
                    op=alu.is_le,
                )
                with nc.allow_low_precision("k <= 50 exact in bf16"):
                    # one reduce for both sides: R counts land in kprep[:,0,0:8],
                    # L in kprep[:,1,0:8] (input tile holds R then L planes)
                    nc.vector.tensor_reduce(
                        out=kprep[:, :, 0:E],
                        in_=cmpf[:],
                        axis=mybir.AxisListType.X, op=alu.add,
                    )

                # expand (k, consts) into edge layout via exact 0/1 matmuls
                for ch in range(8):
                    side = ch % 2  # 0=L, 1=R
                    nc.tensor.matmul(
                        keg_ps[32 * g : 32 * g + 32, ch, :],
                        eabs_bf[:, 32 * ch : 32 * ch + 32],
                        kprep[:, 1 - side, :],
                        start=True, stop=True, tile_position=(0, 32 * g),
                    )
                nc.scalar.copy(
                    keg[32 * g : 32 * g + 32, :, :], keg_ps[32 * g : 32 * g + 32, :, :]
                )

            # passes delayed one row: pass g's inputs are already folded when
            # its chain is emitted, so it rarely stalls the fold stream
            for r in range(R):
                do_row(r)
                if r in (2, 4, 6):
                    do_pass(r // 2 - 1)
            do_pass(3)

            # ---- closed-form finals on the edge tile ----
            # R channels (odd) counted crossed steps: k = 50 - count
            nc.vector.tensor_scalar(
                out=keg[:, 1:8:2, 0:E], in0=keg[:, 1:8:2, 0:E], scalar1=-1.0,
                scalar2=float(NSTEPS), op0=alu.mult, op1=alu.add,
            )
            kf = keg[:, :, 0:E]
            adv = keg[:, :, 8]
            bd_b = keg[:, :, 9:10].broadcast_to([128, 8, E])
            adpv = keg[:, :, 10]
            bdp_b = keg[:, :, 11:12].broadcast_to([128, 8, E])

            prep = p_fin.tile([128, 4, 8], f32, tag="prep")
            la, lap, rad, radp = (
                prep[:, 0, :], prep[:, 1, :], prep[:, 2, :], prep[:, 3, :],
            )
            t8 = p_fin.tile([128, 8], f32, tag="t8")

            def ln1p(out, x):  # ln(1+x) ~ x*(1 - x/2), |x| <= ~2e-3
                nc.vector.tensor_scalar(
                    out=t8[:], in0=x, scalar1=-0.5, scalar2=1.0,
                    op0=alu.mult, op1=alu.add,
                )
                nc.vector.tensor_tensor(out=out, in0=t8[:], in1=x, op=alu.mult)

            ln1p(la, adv)
            ln1p(lap, adpv)
            nc.vector.reciprocal(rad, adv)
            nc.vector.reciprocal(radp, adpv)

            def bview(x, n=E):  # [128, m] -> [128, m, n] broadcast
                return x.rearrange("p (c o) -> p c o", o=1).broadcast_to(
                    [128, x.shape[1], n]
                )

            tt = nc.vector.tensor_tensor
            ts = nc.vector.tensor_scalar

            def expm1s(out, y, tmp):  # y*(1+y/2*(1+y/3*(1+y/4))), |y| <= ~0.1
                ts(out=tmp[:], in0=y[:], scalar1=0.25, scalar2=1.0,
                   op0=alu.mult, op1=alu.add)
                tt(out=tmp[:], in0=tmp[:], in1=y[:], op=alu.mult)
                ts(out=tmp[:], in0=tmp[:], scalar1=1.0 / 3.0, scalar2=1.0,
                   op0=alu.mult, op1=alu.add)
                tt(out=tmp[:], in0=tmp[:], in1=y[:], op=alu.mult)
                ts(out=tmp[:], in0=tmp[:], scalar1=0.5, scalar2=1.0,
                   op0=alu.mult, op1=alu.add)
                tt(out=out[:], in0=tmp[:], in1=y[:], op=alu.mult)

            # x0 views from the grow-layout grid constant (4D; strided views
            # cannot be flattened, so edge/bulk ops run on 4D access patterns)
            x0e = x0g_sb[:, :, 0:8:7, :]
            x0bulk = x0g_sb[:, :, 1:7, :]

            def v4(a):  # [128, 8, E] contiguous tile -> [128, 4, 2, E] view
                return a.rearrange("p (c f) e -> p c f e", f=2)

            y = p_fin.tile([128, 8, E], f32, tag="y")
            tmp = p_fin.tile([128, 8, E], f32, tag="tmp")
            em = p_fin.tile([128, 8, E], f32, tag="em")
            emp = p_fin.tile([128, 8, E], f32, tag="emp")
            # em chain on DVE; em' chain on GpSimd in parallel
            tt(out=y[:], in0=kf, in1=bview(la), op=alu.mult)
            expm1s(em, y, tmp)
            kc = p_fin.tile([128, 8, E], f32, tag="kc")
            y2 = p_fin.tile([128, 8, E], f32, tag="y2")
            tm2 = p_fin.tile([128, 8, E], f32, tag="tm2")
            spb = p_fin.tile([128, 8, E], f32, tag="spb")
            gt_ = nc.gpsimd.tensor_tensor
            gs_ = nc.gpsimd.tensor_scalar
            gs_(out=kc[:], in0=kf, scalar1=-1.0, scalar2=float(NSTEPS),
                op0=alu.mult, op1=alu.add)
            gt_(out=y2[:], in0=kc[:], in1=bview(lap), op=alu.mult)
            gs_(out=tm2[:], in0=y2[:], scalar1=0.25, scalar2=1.0,
                op0=alu.mult, op1=alu.add)
            gt_(out=tm2[:], in0=tm2[:], in1=y2[:], op=alu.mult)
            gs_(out=tm2[:], in0=tm2[:], scalar1=1.0 / 3.0, scalar2=1.0,
                op0=alu.mult, op1=alu.add)
            gt_(out=tm2[:], in0=tm2[:], in1=y2[:], op=alu.mult)
            gs_(out=tm2[:], in0=tm2[:], scalar1=0.5, scalar2=1.0,
                op0=alu.mult, op1=alu.add)
            gt_(out=emp[:], in0=tm2[:], in1=y2[:], op=alu.mult)
            gt_(out=spb[:], in0=emp[:], in1=bview(radp), op=alu.mult)
            gt_(out=spb[:], in0=spb[:], in1=bdp_b, op=alu.mult)  # S'*bd'

            grow = p_fin.tile([128, 4, 8, E], f32, tag="grow")
            growe = grow[:, :, 0:8:7, :]

            # x50 = (1+em')*((1+em)*x0 + em*rad*bd) + em'*radp*bd'
            P = p_fin.tile([128, 8, E], f32, tag="P")
            tt(out=tmp[:], in0=em[:], in1=bview(rad), op=alu.mult)
            tt(out=tmp[:], in0=tmp[:], in1=bd_b, op=alu.mult)  # S*bd
            tt(out=v4(y[:]), in0=v4(em[:]), in1=x0e, op=alu.mult)
            tt(out=tmp[:], in0=tmp[:], in1=y[:], op=alu.add)
            tt(out=v4(P[:]), in0=v4(tmp[:]), in1=x0e, op=alu.add)  # u*x0 + S*bd
            tt(out=tmp[:], in0=emp[:], in1=P[:], op=alu.mult)
            tt(out=tmp[:], in0=tmp[:], in1=P[:], op=alu.add)  # u'*P
            tt(out=growe, in0=v4(tmp[:]), in1=v4(spb[:]), op=alu.add)

            # bulk = k=50 case per cell: x = (1+em50)*x0 + em50*rad*bd
            la4 = la[:, 0:8:2]
            rad4 = rad[:, 0:8:2]
            bd4 = keg[:, 0:8:2, 9]
            t4 = p_fin.tile([128, 4], f32, tag="t4")
            y4 = p_fin.tile([128, 4], f32, tag="y4")
            em50 = p_fin.tile([128, 4], f32, tag="em50")
            gs_(out=y4[:], in0=la4, scalar1=float(NSTEPS), scalar2=None, op0=alu.mult)
            gs_(out=t4[:], in0=y4[:], scalar1=0.25, scalar2=1.0,
                op0=alu.mult, op1=alu.add)
            gt_(out=t4[:], in0=t4[:], in1=y4[:], op=alu.mult)
            gs_(out=t4[:], in0=t4[:], scalar1=1.0 / 3.0, scalar2=1.0,
                op0=alu.mult, op1=alu.add)
            gt_(out=t4[:], in0=t4[:], in1=y4[:], op=alu.mult)
            gs_(out=t4[:], in0=t4[:], scalar1=0.5, scalar2=1.0,
                op0=alu.mult, op1=alu.add)
            gt_(out=em50[:], in0=t4[:], in1=y4[:], op=alu.mult)
            sbd4 = p_fin.tile([128, 4], f32, tag="sbd4")
            gt_(out=sbd4[:], in0=em50[:], in1=rad4, op=alu.mult)
            gt_(out=sbd4[:], in0=sbd4[:], in1=bd4, op=alu.mult)
            growb = grow[:, :, 1:7, :]

            def b4(x):  # [128, 4] -> [128, 4, 6, E] broadcast
                return x.rearrange("p (c o u) -> p c o u", o=1, u=1).broadcast_to(
                    [128, 4, 6, E]
                )

            tb = p_fin.tile([128, 4, 6, E], f32, tag="tb")
            gt_(out=tb[:], in0=x0bulk, in1=b4(em50[:]), op=alu.mult)
            gt_(out=tb[:], in0=tb[:], in1=x0bulk, op=alu.add)
            gt_(out=growb, in0=tb[:], in1=b4(sbd4[:]), op=alu.add)

            # ---- one contiguous store: grow == gamma rows (sync queue is
            # empty by now; scalar queue still drains const traffic) ----
            nc.sync.dma_start(
                gamma.rearrange("r (q m) -> (r q) m", m=4 * 64),
                grow[:].rearrange("p c f e -> p (c f e)"),
            )

    nc.compile()
    return nc


def _host_constants():
    f32 = np.float32
    grid = np.linspace(0.0, 1.0, S).astype(f32)
    c = np.arange(128, dtype=np.int64) % 64
    x0map = grid[(64 * c)[:, None] + np.arange(64)[None, :]]
    # grow layout: x0g[p, 64*c4 + j] = grid[64*(4*(p%16)+c4) + j]
    cq = np.arange(128, dtype=np.int64) % 16
    cell = 4 * cq[:, None] + np.arange(256)[None, :] // 64
    x0g = grid[64 * cell + np.arange(256)[None, :] % 64]
    tknots = np.stack([c / 64.0, (c + 1) / 64.0], axis=1).astype(f32)
    sel = np.zeros((128, 6 * 64), dtype=f32)
    cc = np.arange(64)
    sel[2 * cc, 0 * 64 + cc] = 1.0  # a_cur
    sel[2 * cc + 1, 1 * 64 + cc] = 1.0  # b_cur
    sel[np.minimum(2 * cc + 2, 126), 2 * 64 + cc] = 1.0  # a_nxt (c=63 -> self)
    sel[np.maximum(2 * cc - 2, 0), 3 * 64 + cc] = 1.0  # a_prv (c=0 -> self)
    sel[np.minimum(2 * cc + 3, 127), 4 * 64 + cc] = 1.0  # b_nxt (c=63 -> self)
    sel[np.maximum(2 * cc - 1, 1), 5 * 64 + cc] = 1.0  # b_prv (c=0 -> self)
    onesS = np.full((128, 1), 1.0 / S, dtype=f32)  # 2^-12, exact

    # expansion selectors: k = h*64 + c (pass layout), m = 16*h + cq (local)
    eabs = np.zeros((128, 8 * 32), dtype=f32)
    for ch in range(8):
        c4 = ch // 2
        for m in range(32):
            h, cq_ = m // 16, m % 16
            k = h * 64 + 4 * cq_ + c4
            eabs[k, 32 * ch + m] = 1.0
    return x0map, x0g, tknots, sel, onesS, eabs


def _in_map(input_seq_slice, W_loc, b_loc, basis, consts):
    f32 = np.float32
    x0map, x0g, tknots, sel, onesS, eabs = consts
    # fold theta->A->selector gathers into one matrix: cons_q = WBS_q^T mean + bc_q
    WB = np.asarray(W_loc, dtype=np.float64) @ np.asarray(basis, dtype=np.float64).T
    bb = np.asarray(b_loc, dtype=np.float64) @ np.asarray(basis, dtype=np.float64).T
    wbs = (WB @ sel.astype(np.float64).reshape(128, 6, 64).transpose(1, 0, 2)).transpose(
        1, 0, 2
    )  # [128, 6, 64]
    bcons_c = bb @ sel.astype(np.float64).reshape(128, 6, 64).transpose(1, 0, 2)  # [6, 64]
    bcons = np.tile(bcons_c.T, (2, 1)).astype(f32)  # [128=(h c), 6]
    return {
        "seq": np.ascontiguousarray(input_seq_slice, dtype=f32),
        "pk": np.ascontiguousarray(
            np.concatenate(
                [
                    x0map, x0g, tknots,
                    wbs.reshape(128, 384).astype(f32), onesS, eabs, bcons,
                ],
                axis=1,
            ).astype(f32)
        ),
    }


def kernel(input_seq, W_loc, b_loc, basis):
    from concourse.bass_utils import run_bass_kernel_spmd

    if "nc" not in _CACHE:
        _CACHE["nc"] = _build_program()
    nc = _CACHE["nc"]
    consts = _host_constants()
    in_maps = [
        _in_map(input_seq[k * R : (k + 1) * R], W_loc, b_loc, basis, consts)
        for k in range(NCORES)
    ]
    res = run_bass_kernel_spmd(nc, in_maps, core_ids=list(range(NCORES)))
    return np.concatenate([r["gamma"] for r in res.results], axis=0)
